# revision 1
# baseline (speedup 1.0000x reference)
"""Trainium2 Bass kernel: PQ-style retrieval argmax over 100k hubs.

Problem: energy[b, n] = sum_c <x[b, c, :], codebooks[c, wiring[n, c], :]>,
output argmax_n energy[b, n] for each of B=2048 rows.

Strategy (database-parallel over 8 cores, 12500 hubs each):
  - host: transpose X -> XT [384, 2048], flatten codebooks -> [2048, 48],
    combine wiring indices to c*256 + w -> per-core [128, 98*8] i32.
  - device, per super-block of 1536 hubs:
      gather hub leaf rows (indirect DMA, 192B records) -> H [128, 384]
      PE-transpose -> HT [384, super] (d on partitions)
      for each of 16 batch tiles: 3x3 matmuls accumulate energy [128, 1536]
      DVE max8 + max_index per (b_tile, super) -> winner (val, idx) tables
  - final: strided max8 over per-super winners, one-hot select index,
    outputs per-core (val [128,16], idx [128,16]); host reduces 8 cores.
"""

import numpy as np

import concourse.bacc as bacc
import concourse.mybir as mybir
import concourse.tile as tile
from concourse.bass import IndirectOffsetOnAxis
from concourse.bass_utils import run_bass_kernel_spmd
from concourse.masks import make_identity

B = 2048          # batch rows
C = 8             # chunks
KCB = 256         # codebook entries per chunk
d = 48            # sub dim
D = C * d         # 384
NCORES = 8
NHUBS = 100000
NL = NHUBS // NCORES   # 12500 hubs per core
NT = 98                # hub tiles of 128 per core (12544)
NLP = NT * 128         # 12544 padded local hubs
SUPER = 1536           # hub super-block (3 PSUM banks)
NSUP = 9               # 8 full supers + 256 tail
TAIL = NLP - (NSUP - 1) * SUPER  # 256
BT = B // 128          # 16 batch tiles
F32 = mybir.dt.float32
I32 = mybir.dt.int32
U32 = mybir.dt.uint32
NEG = -1e30

_cached_nc = None


def _build():
    nc = bacc.Bacc("TRN2", target_bir_lowering=False, debug=False,
                   num_devices=NCORES)
    xt = nc.dram_tensor("xt", [D, B], F32, kind="ExternalInput")
    cb = nc.dram_tensor("cb", [C * KCB, d], F32, kind="ExternalInput")
    idxd = nc.dram_tensor("idx", [128, NT * C], I32, kind="ExternalInput")
    oval = nc.dram_tensor("oval", [128, BT], F32, kind="ExternalOutput")
    oidx = nc.dram_tensor("oidx", [128, BT], I32, kind="ExternalOutput")

    with tile.TileContext(nc) as tc:
        with (
            tc.tile_pool(name="persist", bufs=1) as pp,
            tc.tile_pool(name="h", bufs=4) as hp,
            tc.tile_pool(name="ht", bufs=2) as htp,
            tc.tile_pool(name="fin", bufs=2) as fp,
            tc.tile_pool(name="trp", bufs=2, space="PSUM") as trp,
            tc.tile_pool(name="enp", bufs=2, space="PSUM") as enp,
        ):
            xt_sb = pp.tile([128, 3 * B], F32, tag="xt")
            for k in range(3):
                nc.sync.dma_start(xt_sb[:, k * B:(k + 1) * B],
                                  xt[k * 128:(k + 1) * 128, :])
            idx_sb = pp.tile([128, NT * C], I32, tag="idx")
            nc.sync.dma_start(idx_sb[:], idxd[:])
            ident = pp.tile([128, 128], F32, tag="ident")
            make_identity(nc, ident[:])
            iota16 = pp.tile([128, 16], U32, tag="iota")
            nc.gpsimd.iota(iota16[:], pattern=[[1, 16]], base=0,
                           channel_multiplier=0)
            # winner tables: per (b_tile, super) an 8-wide max8/max_index slot
            wv = pp.tile([128, BT * 16 * 8], F32, tag="wv")
            wi = pp.tile([128, BT * 16 * 8], U32, tag="wi")
            nc.gpsimd.memset(wv[:], NEG)
            nc.gpsimd.memset(wi[:], 0)
            ovs = pp.tile([128, BT], F32, tag="ovs")
            ois = pp.tile([128, BT], I32, tag="ois")

            for s in range(NSUP):
                S = SUPER if s < NSUP - 1 else TAIL
                ht = htp.tile([128, 3 * SUPER], F32, tag="ht")
                for t in range(S // 128):
                    tt = s * (SUPER // 128) + t
                    h = hp.tile([128, D], F32, tag="h")
                    for c in range(C):
                        col = tt * C + c
                        nc.gpsimd.indirect_dma_start(
                            out=h[:, c * d:(c + 1) * d],
                            out_offset=None,
                            in_=cb[:, :],
                            in_offset=IndirectOffsetOnAxis(
                                ap=idx_sb[:, col:col + 1], axis=0),
                        )
                    tr = trp.tile([128, D], F32, tag="tr")
                    for k in range(3):
                        nc.tensor.transpose(out=tr[:, k * 128:(k + 1) * 128],
                                            in_=h[:, k * 128:(k + 1) * 128],
                                            identity=ident[:])
                    # scatter the 3 k-slices into ht at column t*128
                    dst = ht[:].rearrange("p (k x) -> p k x", k=3)[
                        :, :, t * 128:(t + 1) * 128]
                    nc.scalar.copy(out=dst, in_=tr[:])

                for b in range(BT):
                    en = enp.tile([128, SUPER], F32, tag="en")
                    for n0 in range(0, S, 512):
                        nw = min(512, S - n0)
                        for k in range(3):
                            nc.tensor.matmul(
                                out=en[:, n0:n0 + nw],
                                lhsT=xt_sb[:, k * B + b * 128:
                                           k * B + (b + 1) * 128],
                                rhs=ht[:, k * SUPER + n0:k * SUPER + n0 + nw],
                                start=(k == 0), stop=(k == 2),
                            )
                    if s == NSUP - 1:
                        # mask the 44 pad hubs (local 12500..12543)
                        pad0 = NL - (NSUP - 1) * SUPER
                        nc.vector.memset(en[:, pad0:S], NEG)
                    w0 = (b * 16 + s) * 8
                    nc.vector.max(out=wv[:, w0:w0 + 8], in_=en[:, :S])
                    nc.vector.max_index(out=wi[:, w0:w0 + 8],
                                        in_max=wv[:, w0:w0 + 8],
                                        in_values=en[:, :S])

            for b in range(BT):
                sv = wv[:, b * 128:(b + 1) * 128:8]   # [128, 16] super winners
                si = wi[:, b * 128:(b + 1) * 128:8]
                gm8 = fp.tile([128, 8], F32, tag="gm8")
                nc.vector.max(out=gm8[:], in_=sv)
                gs8 = fp.tile([128, 8], U32, tag="gs8")
                nc.vector.max_index(out=gs8[:], in_max=gm8[:], in_values=sv)
                oh = fp.tile([128, 16], F32, tag="oh")
                nc.vector.tensor_tensor(
                    out=oh[:], in0=iota16[:],
                    in1=gs8[:, 0:1].to_broadcast([128, 16]),
                    op=mybir.AluOpType.is_equal)
                idxf = fp.tile([128, 16], F32, tag="idxf")
                nc.vector.tensor_copy(out=idxf[:], in_=si)
                prod = fp.tile([128, 16], F32, tag="prod")
                nc.vector.tensor_mul(out=prod[:], in0=idxf[:], in1=oh[:])
                isel = fp.tile([128, 1], F32, tag="isel")
                nc.vector.tensor_reduce(out=isel[:], in_=prod[:],
                                        axis=mybir.AxisListType.X,
                                        op=mybir.AluOpType.add)
                sf = fp.tile([128, 1], F32, tag="sf")
                nc.vector.tensor_copy(out=sf[:], in_=gs8[:, 0:1])
                nc.vector.tensor_scalar_mul(sf[:], sf[:], float(SUPER))
                nc.vector.tensor_add(out=sf[:], in0=sf[:], in1=isel[:])
                nc.vector.tensor_copy(out=ois[:, b:b + 1], in_=sf[:])
                nc.vector.tensor_copy(out=ovs[:, b:b + 1], in_=gm8[:, 0:1])

            nc.sync.dma_start(oval[:], ovs[:])
            nc.sync.dma_start(oidx[:], ois[:])

    nc.compile()
    return nc


def _get_nc():
    global _cached_nc
    if _cached_nc is None:
        _cached_nc = _build()
    return _cached_nc


def make_in_maps(input_features, codebooks, wiring, mask):
    x = np.asarray(input_features, dtype=np.float32)
    cbk = np.asarray(codebooks, dtype=np.float32)
    w = np.asarray(wiring).astype(np.int32)
    m = np.asarray(mask).astype(np.float32)
    xm = x * np.repeat(m, d)[None, :]
    xt = np.ascontiguousarray(xm.T)                       # [384, 2048]
    cbf = np.ascontiguousarray(cbk.reshape(C * KCB, d))   # [2048, 48]
    chunk_off = (np.arange(C, dtype=np.int32) * KCB)[None, :]
    in_maps = []
    for mc in range(NCORES):
        wp = np.zeros((NLP, C), np.int32)
        wp[:NL] = w[mc * NL:(mc + 1) * NL]
        comb = wp + chunk_off
        idx_arr = np.ascontiguousarray(
            comb.reshape(NT, 128, C).transpose(1, 0, 2).reshape(128, NT * C))
        in_maps.append({"xt": xt, "cb": cbf, "idx": idx_arr})
    return in_maps


def reduce_outputs(results):
    vals = np.stack([r["oval"].T.reshape(B) for r in results])  # [8, 2048]
    idxs = np.stack([r["oidx"].T.reshape(B) for r in results])
    win = np.argmax(vals, axis=0)
    out = win.astype(np.int64) * NL + idxs[win, np.arange(B)]
    return out.astype(np.int32)


def kernel(**inputs):
    nc = _get_nc()
    in_maps = make_in_maps(inputs["input_features"], inputs["codebooks"],
                           inputs["wiring"], inputs["mask"])
    res = run_bass_kernel_spmd(nc, in_maps, core_ids=list(range(NCORES)))
    return reduce_outputs(res.results)
